# revision 21
# baseline (speedup 1.0000x reference)
"""GCN 2-layer message-passing kernel for 8 trn2 NeuronCores.

Strategy (graph-parallel by dst-node range, per sharding hint):
  - Nodes sharded 8 ways by dst range. Each core aggregates its in-edges.
  - Aggregation agg[d] = sum_e norm_e * table[src_e] is done as:
      dma_gather (MoE gather primitive) of source rows into SBUF, then
      TensorE matmul against a host-built one-hot-times-norm matrix S
      (segmented sum), accumulating in PSUM per 64-dst window, RMW-added
      into a transposed SBUF accumulator at a register-dynamic offset.
  - Layer 1 aggregates X directly (aggregate-first), then applies W1 on
    the core's node slice (transposed orientation feeds W matmuls with
    per-partition bias+relu on ScalarE), then W2 -> T2 = relu(.)@W2.
  - Host gathers per-core T2 slices into the full layer-2 table between
    launches; layer 2 aggregates T2, applies bias+relu, writes out.
  - int16 gather indices cap at 32767, so each core's edges split into a
    "low" stream (src < 32768) and "high" stream (src >= 32768) gathering
    from the two halves of the table.
"""

import ml_dtypes
import numpy as np

import concourse.bass as bass
import concourse.bacc as bacc
import concourse.mybir as mybir
from concourse.tile import TileContext
from concourse.bass_utils import run_bass_kernel_spmd

F32 = mybir.dt.float32
I16 = mybir.dt.int16
I32 = mybir.dt.int32

# bf16 gather tables + S matrices: halves the dominant gather traffic and
# runs the segmented-sum matmuls at 1 cyc/row (fp32 is 4). PSUM accumulation
# and the dense W1/W2 transforms stay fp32.
USE_BF16 = True
MSG_DT = mybir.dt.bfloat16 if USE_BF16 else F32
MSG_NP = ml_dtypes.bfloat16 if USE_BF16 else np.float32

NCORES = 8
CH = 128          # tokens per chunk (matmul contraction)
LO_G = 4          # chunks per window, low stream
HI_G = 2          # chunks per window, high stream
SPAN = 64         # max dst span per window (S columns)
BATCH = 16        # chunks per gather call


# ---------------------------------------------------------------- host side

def _pack_stream(src, dstl, norm, g):
    """Pack one dst-sorted token stream into windows of g*CH tokens with
    dst span < SPAN. Returns (src_pad, col_pad, norm_pad, bases)."""
    wt = g * CH
    T = len(src)
    o_src, o_col, o_nrm, bases = [], [], [], []
    pos = 0
    while pos < T:
        base = int(dstl[pos])
        end = min(pos + wt, T)
        # tokens beyond span limit go to the next window
        v = int(np.searchsorted(dstl[pos:end], base + SPAN))
        take = v
        s = np.zeros(wt, dtype=np.int16)
        c = np.zeros(wt, dtype=np.int64)
        n = np.zeros(wt, dtype=np.float32)
        s[:take] = src[pos:pos + take]
        c[:take] = dstl[pos:pos + take] - base
        n[:take] = norm[pos:pos + take]
        o_src.append(s); o_col.append(c); o_nrm.append(n)
        bases.append(base)
        pos += take
    if not bases:
        o_src.append(np.zeros(wt, np.int16))
        o_col.append(np.zeros(wt, np.int64))
        o_nrm.append(np.zeros(wt, np.float32))
        bases.append(0)
    return (np.concatenate(o_src), np.concatenate(o_col),
            np.concatenate(o_nrm), np.array(bases, dtype=np.int32))


def _pad_windows(src, col, nrm, bases, g, n_win_target):
    wt = g * CH
    cur = len(bases)
    if cur < n_win_target:
        extra = n_win_target - cur
        src = np.concatenate([src, np.zeros(extra * wt, np.int16)])
        col = np.concatenate([col, np.zeros(extra * wt, np.int64)])
        nrm = np.concatenate([nrm, np.zeros(extra * wt, np.float32)])
        bases = np.concatenate([bases, np.zeros(extra, np.int32)])
    return src, col, nrm, bases


def _stream_arrays(src, col, nrm):
    """Device layouts: idx [128, K*8] int16 (replicated), S [128, K*64] f32."""
    T = len(src)
    K = T // CH
    t = np.arange(T)
    # idx: token t -> [t%16, 8*(t//128) + (t%128)//16]
    idx = np.zeros((16, K * 8), dtype=np.int16)
    idx[t % 16, 8 * (t // CH) + (t % CH) // 16] = src
    idx = np.tile(idx, (8, 1))
    # S: token t -> [t%128, 64*(t//128) + col]
    S = np.zeros((CH, K * SPAN), dtype=np.float32)
    S[t % CH, SPAN * (t // CH) + col] = nrm
    return idx, S.astype(MSG_NP)


def _preprocess(x, edge_index, n, npc, split):
    e_src = edge_index[0].astype(np.int64)
    e_dst = edge_index[1].astype(np.int64)
    loop = np.arange(n, dtype=np.int64)
    src_all = np.concatenate([e_src, loop])
    dst_all = np.concatenate([e_dst, loop])
    deg = np.bincount(dst_all, minlength=n).astype(np.float32)
    dinv = (1.0 / np.sqrt(np.maximum(deg, 1.0))).astype(np.float32)
    norm_all = dinv[src_all] * dinv[dst_all]

    per_core = []
    for c in range(NCORES):
        sel = (dst_all >= c * npc) & (dst_all < (c + 1) * npc)
        s, d, nm = src_all[sel], dst_all[sel] - c * npc, norm_all[sel]
        order = np.argsort(d, kind="stable")
        s, d, nm = s[order], d[order], nm[order]
        lo_sel = s < split
        lo = _pack_stream(s[lo_sel].astype(np.int16), d[lo_sel], nm[lo_sel], LO_G)
        hi_m = ~lo_sel
        hi = _pack_stream((s[hi_m] - split).astype(np.int16), d[hi_m], nm[hi_m], HI_G)
        per_core.append((lo, hi))

    def round_to(v, m):
        return ((v + m - 1) // m) * m

    # common window counts (batches hold whole windows)
    nwl = round_to(max(len(pc[0][3]) for pc in per_core), BATCH // LO_G)
    nwh = round_to(max(len(pc[1][3]) for pc in per_core), BATCH // HI_G)

    metas = []
    for c in range(NCORES):
        lo = _pad_windows(*per_core[c][0], LO_G, nwl)
        hi = _pad_windows(*per_core[c][1], HI_G, nwh)
        idx_lo, S_lo = _stream_arrays(lo[0], lo[1], lo[2])
        idx_hi, S_hi = _stream_arrays(hi[0], hi[1], hi[2])
        S = np.concatenate([S_lo, S_hi], axis=1)
        bases = np.concatenate([lo[3], hi[3]])[None, :].astype(np.int32)
        metas.append(dict(idx_lo=idx_lo, idx_hi=idx_hi, S=S, bases=bases))
    return metas, nwl, nwh


# -------------------------------------------------------------- device side

def _segsum(nc, tc, pools, table_los, table_his, fin, nwl, nwh, aggt, npad,
            idx_lo_t, idx_hi_t, s_t, bases_sb, breg):
    """Emit gather + segmented-sum for both streams.

    aggt: SBUF tile [128, nfh*npad]; fin = table feature width (128*nfh).
    """
    gpool, spool, ipool, ppool = pools
    nfh = fin // 128
    kglob = 0
    wglob = 0
    for tables, nw, g, idx_t in ((table_los, nwl, LO_G, idx_lo_t),
                                 (table_his, nwh, HI_G, idx_hi_t)):
        kcnt = nw * g
        nb = kcnt // BATCH
        win_per_b = BATCH // g
        for b in range(nb):
            gt = gpool.tile([128, BATCH * fin], MSG_DT, tag="gt")
            st = spool.tile([128, BATCH * SPAN], MSG_DT, tag="st")
            it = ipool.tile([128, BATCH * 8], I16, tag="it")
            nc.sync.dma_start(st[:], s_t[:, (kglob + b * BATCH) * SPAN:
                                         (kglob + (b + 1) * BATCH) * SPAN])
            nc.sync.dma_start(it[:], idx_t[:, b * BATCH * 8:(b + 1) * BATCH * 8])
            gt3 = gt[:].rearrange("p (b e) -> p b e", e=fin)
            # >1024 tokens per gather call exceeds the SWDGE packet limit
            for j0 in range(0, BATCH, 8):
                nc.gpsimd.dma_gather(gt3[:, j0:j0 + 8, :], tables,
                                     it[:, j0 * 8:(j0 + 8) * 8],
                                     8 * CH, 8 * CH, fin)
            for wi in range(win_per_b):
                w = wglob + b * win_per_b + wi
                pts = [ppool.tile([128, SPAN], F32, tag=f"ps{fh}",
                                  name=f"ps{fh}") for fh in range(nfh)]
                for j0 in range(g):
                    j = wi * g + j0
                    for fh in range(nfh):
                        nc.tensor.matmul(
                            pts[fh][:],
                            lhsT=gt[:, j * fin + fh * 128:j * fin + fh * 128 + 128],
                            rhs=st[:, j * SPAN:(j + 1) * SPAN],
                            start=(j0 == 0), stop=(j0 == g - 1))
                with tc.tile_critical():
                    nc.vector.reg_load(breg, bases_sb[0:1, w:w + 1])
                    bval = nc.snap(breg, donate=True, min_val=0,
                                   max_val=npad - SPAN)
                    for fh in range(nfh):
                        sl = aggt[:, fh * npad:(fh + 1) * npad]
                        dsl = sl[:, bass.ds(bval, SPAN)]
                        nc.vector.tensor_add(dsl, dsl, pts[fh][:])
        kglob += kcnt
        wglob += nw


def _build_l1(n, f0, f2, npc, split, nwl, nwh):
    nc = bacc.Bacc("TRN2", target_bir_lowering=False)
    npad = npc + SPAN
    kl, kh = nwl * LO_G, nwh * HI_G
    x = nc.dram_tensor("x", [n, f0], MSG_DT, kind="ExternalInput")
    idx_lo = nc.dram_tensor("idx_lo", [128, kl * 8], I16, kind="ExternalInput")
    idx_hi = nc.dram_tensor("idx_hi", [128, kh * 8], I16, kind="ExternalInput")
    s_t = nc.dram_tensor("s", [128, (kl + kh) * SPAN], MSG_DT,
                         kind="ExternalInput")
    bases = nc.dram_tensor("bases", [1, nwl + nwh], I32, kind="ExternalInput")
    w1d = nc.dram_tensor("w1d", [128, 2 * f0], F32, kind="ExternalInput")
    b1d = nc.dram_tensor("b1d", [128, 2], F32, kind="ExternalInput")
    w2d = nc.dram_tensor("w2d", [128, 2 * f2], F32, kind="ExternalInput")
    t2t = nc.dram_tensor("t2t", [128, npc], F32, kind="ExternalOutput")

    with TileContext(nc) as tc:
        with (tc.tile_pool(name="const", bufs=1) as cpool,
              tc.tile_pool(name="gp", bufs=3) as gpool,
              tc.tile_pool(name="sp", bufs=3) as spool,
              tc.tile_pool(name="ip", bufs=3) as ipool,
              tc.tile_pool(name="pp", bufs=2, space="PSUM") as ppool,
              tc.tile_pool(name="px", bufs=2, space="PSUM") as pxpool,
              tc.tile_pool(name="h1p", bufs=2) as h1pool,
              tc.tile_pool(name="op", bufs=3) as opool):
            aggt = cpool.tile([128, 2 * npad], F32)
            nc.vector.memset(aggt[:], 0.0)
            w1sb = cpool.tile([128, 2 * f0], F32)
            nc.sync.dma_start(w1sb[:], w1d[:, :])
            b1sb = cpool.tile([128, 2], F32)
            nc.sync.dma_start(b1sb[:], b1d[:, :])
            w2sb = cpool.tile([128, 2 * f2], F32)
            nc.sync.dma_start(w2sb[:], w2d[:, :])
            bases_sb = cpool.tile([1, nwl + nwh], I32)
            nc.sync.dma_start(bases_sb[:], bases[:, :])
            breg = nc.alloc_register(mybir.EngineType.DVE, "wbase")

            hs = split if split < n else 0
            _segsum(nc, tc, (gpool, spool, ipool, ppool),
                    x[0:split, :], x[hs:n, :], f0, nwl, nwh,
                    aggt, npad, idx_lo, idx_hi, s_t, bases_sb, breg)

            # dense transform: T2.T = W2.T @ relu(W1.T @ AGG1.T + b1)
            ntile = (npc + 127) // 128
            for nt in range(ntile):
                c0 = nt * 128
                w = min(128, npc - c0)
                h1s = []
                for foh in range(2):
                    ps = pxpool.tile([128, w], F32, tag="psA")
                    for kh in range(2):
                        nc.tensor.matmul(
                            ps[:],
                            lhsT=w1sb[:, kh * f0 + foh * 128:kh * f0 + foh * 128 + 128],
                            rhs=aggt[:, kh * npad + c0:kh * npad + c0 + w],
                            start=(kh == 0), stop=(kh == 1))
                    h1 = h1pool.tile([128, w], F32, tag=f"h1{foh}")
                    nc.scalar.activation(h1[:], ps[:],
                                         mybir.ActivationFunctionType.Relu,
                                         bias=b1sb[:, foh:foh + 1], scale=1.0)
                    h1s.append(h1)
                ps2 = pxpool.tile([128, w], F32, tag="psB")
                for foh in range(2):
                    nc.tensor.matmul(ps2[:],
                                     lhsT=w2sb[:, foh * f2:(foh + 1) * f2],
                                     rhs=h1s[foh][:],
                                     start=(foh == 0), stop=(foh == 1))
                o2 = opool.tile([128, w], F32, tag="o2")
                nc.vector.tensor_copy(o2[:], ps2[:])
                nc.sync.dma_start(t2t[:, c0:c0 + w], o2[:])
    nc.finalize()
    return nc


def _build_l2(n, f2, npc, split, nwl, nwh):
    nc = bacc.Bacc("TRN2", target_bir_lowering=False)
    npad = npc + SPAN
    kl, kh = nwl * LO_G, nwh * HI_G
    t2 = nc.dram_tensor("t2", [n, f2], MSG_DT, kind="ExternalInput")
    idx_lo = nc.dram_tensor("idx_lo", [128, kl * 8], I16, kind="ExternalInput")
    idx_hi = nc.dram_tensor("idx_hi", [128, kh * 8], I16, kind="ExternalInput")
    s_t = nc.dram_tensor("s", [128, (kl + kh) * SPAN], MSG_DT,
                         kind="ExternalInput")
    bases = nc.dram_tensor("bases", [1, nwl + nwh], I32, kind="ExternalInput")
    b2d = nc.dram_tensor("b2d", [128, 1], F32, kind="ExternalInput")
    outt = nc.dram_tensor("outt", [128, npc], F32, kind="ExternalOutput")

    with TileContext(nc) as tc:
        with (tc.tile_pool(name="const", bufs=1) as cpool,
              tc.tile_pool(name="gp", bufs=3) as gpool,
              tc.tile_pool(name="sp", bufs=3) as spool,
              tc.tile_pool(name="ip", bufs=3) as ipool,
              tc.tile_pool(name="pp", bufs=2, space="PSUM") as ppool,
              tc.tile_pool(name="op", bufs=3) as opool):
            aggt = cpool.tile([128, npad], F32)
            nc.vector.memset(aggt[:], 0.0)
            b2sb = cpool.tile([128, 1], F32)
            nc.sync.dma_start(b2sb[:], b2d[:, :])
            bases_sb = cpool.tile([1, nwl + nwh], I32)
            nc.sync.dma_start(bases_sb[:], bases[:, :])
            breg = nc.alloc_register(mybir.EngineType.DVE, "wbase")

            hs = split if split < n else 0
            _segsum(nc, tc, (gpool, spool, ipool, ppool),
                    t2[0:split, :], t2[hs:n, :], f2, nwl, nwh,
                    aggt, npad, idx_lo, idx_hi, s_t, bases_sb, breg)

            step = 2048
            for c0 in range(0, npc, step):
                w = min(step, npc - c0)
                ot = opool.tile([128, step], F32, tag="ot")
                nc.scalar.activation(ot[:, :w], aggt[:, c0:c0 + w],
                                     mybir.ActivationFunctionType.Relu,
                                     bias=b2sb[:, 0:1], scale=1.0)
                nc.sync.dma_start(outt[:, c0:c0 + w], ot[:, :w])
    nc.finalize()
    return nc


# ------------------------------------------------------------------- driver

_LAST_EXEC_NS = []


def kernel(x, edge_index, W1, b1, W2, b2, trace=False):
    global _LAST_EXEC_NS
    _LAST_EXEC_NS = []
    x = np.ascontiguousarray(np.asarray(x, dtype=np.float32))
    edge_index = np.asarray(edge_index, dtype=np.int32)
    W1 = np.asarray(W1, dtype=np.float32)
    b1 = np.asarray(b1, dtype=np.float32)
    W2 = np.asarray(W2, dtype=np.float32)
    b2 = np.asarray(b2, dtype=np.float32)

    n, f0 = x.shape
    f2 = W2.shape[1]
    assert n % NCORES == 0
    npc = n // NCORES
    split = min(32768, n)

    metas, nwl, nwh = _preprocess(x, edge_index, n, npc, split)

    w1d = np.ascontiguousarray(
        W1.reshape(2, 128, f0).transpose(1, 0, 2).reshape(128, 2 * f0))
    b1d = np.ascontiguousarray(b1.reshape(2, 128).T)
    w2d = np.ascontiguousarray(
        W2.reshape(2, 128, f2).transpose(1, 0, 2).reshape(128, 2 * f2))
    b2d = np.ascontiguousarray(b2.reshape(f2, 1))

    nc1 = _build_l1(n, f0, f2, npc, split, nwl, nwh)
    xm = np.ascontiguousarray(x.astype(MSG_NP))
    in_maps = []
    for c in range(NCORES):
        m = metas[c]
        in_maps.append(dict(x=xm, idx_lo=m["idx_lo"], idx_hi=m["idx_hi"],
                            s=m["S"], bases=m["bases"], w1d=w1d, b1d=b1d,
                            w2d=w2d))
    res1 = run_bass_kernel_spmd(nc1, in_maps, core_ids=list(range(NCORES)))
    if trace:
        import time as _t
        t0 = _t.time()
        res1 = run_bass_kernel_spmd(nc1, in_maps, core_ids=list(range(NCORES)))
        _LAST_EXEC_NS.append(int((_t.time() - t0) * 1e9))

    t2 = np.concatenate([np.ascontiguousarray(r["t2t"]).T
                         for r in res1.results], axis=0)
    t2 = np.ascontiguousarray(t2.astype(MSG_NP))

    nc2 = _build_l2(n, f2, npc, split, nwl, nwh)
    in_maps2 = []
    for c in range(NCORES):
        m = metas[c]
        in_maps2.append(dict(t2=t2, idx_lo=m["idx_lo"], idx_hi=m["idx_hi"],
                             s=m["S"], bases=m["bases"], b2d=b2d))
    res2 = run_bass_kernel_spmd(nc2, in_maps2, core_ids=list(range(NCORES)))
    if trace:
        import time as _t
        t0 = _t.time()
        res2 = run_bass_kernel_spmd(nc2, in_maps2, core_ids=list(range(NCORES)))
        _LAST_EXEC_NS.append(int((_t.time() - t0) * 1e9))

    out = np.concatenate([np.ascontiguousarray(r["outt"]).T
                          for r in res2.results], axis=0)
    return np.ascontiguousarray(out, dtype=np.float32)
